# revision 8
# baseline (speedup 1.0000x reference)
"""Trainium2 Bass kernel for nn_CAGKE_1 (Gaussian-kernel embedding).

Math: reference computes, for mask m_i = 1[X_i > 0.5],
    out[j] = sum_e softmax(w)_e * sum_i m_i * (c/sigma_e) exp(-(j-i-1)^2/(2 sigma_e^2)) + noise_j
Both sums are linear, so the E=128 Gaussian channels collapse into one
combined kernel ghat(d) = sum_e softmax(w)_e * (c/sigma_e) exp(-d^2/(2 sigma_e^2))
BEFORE the convolution. With sigma in [0.5, 5], the f32 exp underflows to 0
for |d| >= 72, so a 255-tap kernel (|d| <= 127) is exact at f32 precision
(valid up to sigma ~8.8).

Per core (output sharded 1024 columns/core, no collectives):
  1. delta^2 row via f32 iota (exact for |d|<=128) + square
  2. exp table [128 e, 256 d] on ACT with per-partition scale -1/(2 sigma^2)
  3. ghat [1,256] = a^T @ exptable on PE (a_e = exp(w_e - max) * c/sigma_e),
     softmax normalization folded into the PSUM->SBUF copy as *1/Z;
     written into the middle of a zeroed 511-wide buffer
  4. forward banded-Toeplitz fwdG[k,c] = ghat_arr[k+c] via 3 overlapping
     all-positive-stride DMAs (contiguous reads per partition); the index
     reversal the Toeplitz needs lives in the mask's partition order, which
     the host pre-reverses per 128-block (layout only)
  5. out^T[128,8] = 3 accumulating matmuls over diagonal blocks d in {-1,0,1}
  6. noise add fused with the PSUM read, store [128,8]; host de-transposes
"""

import sys

import numpy as np

if "/opt/trn_rl_repo" not in sys.path:
    sys.path.insert(0, "/opt/trn_rl_repo")

T = 8192
E = 128
N_CORES = 8
TJ = T // N_CORES          # 1024 outputs per core
NB = TJ // 128             # 8 output blocks of 128
WIN = TJ + 512             # 1536 = 12 blocks of 128
WIN_BLKS = WIN // 128      # 12
L = 511                    # Toeplitz-expanded taps (|d| <= 255)
LK = 256                   # computed taps, d in [-127, 128]
INV_SQRT_2PI = 0.39894228

_compiled = None


def _build():
    import concourse.bacc as bacc
    import concourse.bass as bass
    import concourse.mybir as mybir
    import concourse.tile as tile

    f32 = mybir.dt.float32
    nc = bacc.Bacc(num_devices=N_CORES, debug=False)

    xw_d = nc.dram_tensor("xw", [128, WIN_BLKS], f32, kind="ExternalInput")
    sg_d = nc.dram_tensor("sigma", [E], f32, kind="ExternalInput")
    w_d = nc.dram_tensor("weight", [1, E], f32, kind="ExternalInput")
    nz_d = nc.dram_tensor("noisew", [128, NB], f32, kind="ExternalInput")
    out_d = nc.dram_tensor("out", [128, NB], f32, kind="ExternalOutput")
    ghat_d = nc.dram_tensor("ghat_scratch", [L], f32, kind="Internal")

    with tile.TileContext(nc) as tc:
        with (
            tc.tile_pool(name="pool", bufs=1) as pool,
            tc.tile_pool(name="psum", bufs=1, space="PSUM") as psum,
        ):
            # ---- latency-critical input loads first ----
            w = pool.tile([1, E], f32, tag="w")
            nc.sync.dma_start(w[:], w_d[:])
            sgr = pool.tile([1, E], f32, tag="sgr")
            nc.sync.dma_start(sgr[:], bass.AP(sg_d, 0, [[1, 1], [1, E]]))
            xT = pool.tile([128, WIN_BLKS], f32, tag="xT")
            nc.sync.dma_start(xT[:], xw_d[:])
            nzT = pool.tile([128, NB], f32, tag="nzT")
            nc.sync.dma_start(nzT[:], nz_d[:])

            # ---- input-independent prep ----
            # dummy Exp forces the ACT exp-table load (~1.3us) off the
            # critical path, before the softmax exp needs it
            dum = pool.tile([1, 1], f32, tag="dum")
            nc.vector.memset(dum[:], 0.0)
            nc.scalar.activation(dum[:], dum[:], mybir.ActivationFunctionType.Exp)
            ghat_s = pool.tile([1, L + 1], f32, tag="ghat_s")
            nc.vector.memset(ghat_s[:], 0.0)
            # zero wings of ghat_d land in DRAM early; the middle is
            # overwritten later, so the Toeplitz chunk loads only wait on
            # the small middle store
            nc.sync.dma_start(ghat_d[:], ghat_s[:, :L])
            dlt = pool.tile([128, LK], f32, tag="dlt")
            nc.gpsimd.iota(
                dlt[:], pattern=[[1, LK]], base=-127, channel_multiplier=0,
                allow_small_or_imprecise_dtypes=True,
            )
            d2 = pool.tile([128, LK], f32, tag="d2")
            nc.vector.tensor_mul(d2[:], dlt[:], dlt[:])

            # ---- binarize mask (host already block-reversed the layout) ----
            mT = pool.tile([128, WIN_BLKS], f32, tag="mT")
            nc.vector.tensor_scalar(mT[:], xT[:], 0.5, None, mybir.AluOpType.is_gt)

            # ---- softmax numerator on one partition; Z folded in later ----
            nmx = pool.tile([1, 1], f32, tag="nmx")
            nc.vector.tensor_reduce(
                nmx[:], w[:], axis=mybir.AxisListType.X, op=mybir.AluOpType.max,
                negate=True,
            )
            ex = pool.tile([1, E], f32, tag="ex")
            nc.scalar.activation(
                ex[:], w[:], mybir.ActivationFunctionType.Exp, bias=nmx[:], scale=1.0
            )
            sm = pool.tile([1, 1], f32, tag="sm")
            nc.vector.tensor_reduce(
                sm[:], ex[:], axis=mybir.AxisListType.X, op=mybir.AluOpType.add
            )
            rz = pool.tile([1, 1], f32, tag="rz")
            nc.vector.reciprocal(rz[:], sm[:])

            # ---- a-row and scale-row computed BEFORE the transposes ----
            s2r = pool.tile([1, E], f32, tag="s2r")
            nc.vector.tensor_mul(s2r[:], sgr[:], sgr[:])
            nc.vector.tensor_scalar_mul(s2r[:], s2r[:], -2.0)
            invs_r = pool.tile([1, E], f32, tag="invs_r")
            nc.vector.reciprocal(invs_r[:], s2r[:])       # -1/(2 sigma^2) row
            rsr = pool.tile([1, E], f32, tag="rsr")
            nc.vector.reciprocal(rsr[:], sgr[:])          # 1/sigma row
            a_r = pool.tile([1, E], f32, tag="a_r")
            nc.vector.tensor_mul(a_r[:], ex[:], rsr[:])
            nc.vector.tensor_scalar_mul(a_r[:], a_r[:], INV_SQRT_2PI)

            # ---- transpose a-row and invs-row -> per-partition columns ----
            ident1 = pool.tile([1, 1], f32, tag="ident1")
            nc.any.memset(ident1[:], 1.0)
            ttp = psum.tile([128, 2], f32, tag="ttp")
            nc.tensor.transpose(ttp[:, 0:1], a_r[:], ident1[:])
            nc.tensor.transpose(ttp[:, 1:2], invs_r[:], ident1[:])
            tt = pool.tile([128, 2], f32, tag="tt")
            nc.vector.tensor_copy(tt[:], ttp[:])
            a = tt[:, 0:1]
            invs = tt[:, 1:2]

            # ---- exp table and combined kernel ghat ----
            expt = pool.tile([128, LK], f32, tag="expt")
            nc.scalar.activation(
                expt[:], d2[:], mybir.ActivationFunctionType.Exp, scale=invs
            )
            ghat_p = psum.tile([1, LK], f32, tag="ghat_p")
            nc.tensor.matmul(ghat_p[:], a, expt[:], start=True, stop=True)
            # 1/Z applied during the PSUM->SBUF copy; lands at arr[128:384]
            nc.vector.tensor_scalar_mul(ghat_s[:, 128 : 128 + LK], ghat_p[:], rz[:])
            nc.sync.dma_start(
                bass.AP(ghat_d, 128, [[1, 1], [1, LK]]), ghat_s[:, 128 : 128 + LK]
            )

            # ---- forward banded Toeplitz, 3 chunks pipelined with matmuls ----
            # fwdG[k, 128q + cr] = ghat_arr[k + 128q + cr]
            op = psum.tile([128, NB], f32, tag="op")
            fwdG = pool.tile([128, 3 * 128], f32, tag="fwdG")
            for q in range(3):
                nc.sync.dma_start(
                    fwdG[:, 128 * q : 128 * (q + 1)],
                    bass.AP(ghat_d, 128 * q, [[1, 128], [1, 128]]),
                )
            for q in range(3):
                d = q - 1
                # Out[ur, ub] += sum_k fwdG[k, 128q+ur] * mT[k, ub+2-d]
                nc.tensor.matmul(
                    op[:],
                    fwdG[:, 128 * q : 128 * (q + 1)],
                    mT[:, 2 - d : 2 - d + NB],
                    start=(q == 0),
                    stop=(q == 2),
                )

            # ---- add noise (fused with PSUM read), store ----
            outS = pool.tile([128, NB], f32, tag="outS")
            nc.vector.tensor_add(outS[:], op[:], nzT[:])
            nc.sync.dma_start(out_d[:], outS[:])

    nc.compile()
    return nc


def kernel(X, sigma, weight, noise):
    global _compiled
    from concourse.bass_utils import run_bass_kernel_spmd

    X = np.ascontiguousarray(np.asarray(X, dtype=np.float32)).reshape(1, T)
    sigma = np.ascontiguousarray(np.asarray(sigma, dtype=np.float32)).reshape(E)
    weight = np.ascontiguousarray(np.asarray(weight, dtype=np.float32)).reshape(1, E)
    noise = np.ascontiguousarray(np.asarray(noise, dtype=np.float32)).reshape(1, T)

    if _compiled is None:
        _compiled = _build()
    nc = _compiled

    # reference coefficient is ghat(j-i-1): the +257 (not +256) pad realizes
    # the -1 shift so the device's Toeplitz can use ghat(u-v)
    Xp = np.zeros(T + 512, dtype=np.float32)
    Xp[257 : 257 + T] = X[0]
    in_maps = []
    for c in range(N_CORES):
        # window [12 blocks x 128], reversed within each block, -> [128, 12]
        wc = Xp[c * TJ : c * TJ + WIN].reshape(WIN_BLKS, 128)[:, ::-1].T
        nzc = noise[0, c * TJ : (c + 1) * TJ].reshape(NB, 128).T
        in_maps.append(
            {
                "xw": np.ascontiguousarray(wc),
                "sigma": sigma,
                "weight": weight,
                "noisew": np.ascontiguousarray(nzc),
            }
        )

    res = run_bass_kernel_spmd(nc, in_maps, core_ids=list(range(N_CORES)))
    out = np.empty((1, T), dtype=np.float32)
    for c in range(N_CORES):
        out[0, c * TJ : (c + 1) * TJ] = res.results[c]["out"].T.reshape(-1)
    return out
